# revision 2
# baseline (speedup 1.0000x reference)
"""Block-causal GQA attention for Trainium2, 8 NeuronCores.

Sharding: core = (batch b, GQA group g): 2 batches x 4 kv-groups.
Each core computes its 4 q-heads + 1 kv-head on one batch element in a
"transposed" layout (head_dim on partitions, tokens on free dim), then a
row-parallel partial out-projection; the host sums the 4 partials per batch.

Layout/algebra notes:
- All matmuls run in float32r (full-rate fp32 on the PE at free-dim >= 256).
- RMSNorm weights + attention scale are folded into host-built RoPE tables;
  rotate_half becomes a partition-block swap (sign folded into the sin table).
- 1/rms factors are applied via gpsimd partition_broadcast + one DVE multiply.
- Softmax: scores never need a running max (|s| bounded via host-computed C
  shift); exp on ScalarE reads PSUM directly; denominator comes free as a
  65th ones-row on V in the PV matmul; normalization fuses into the PSUM
  evacuation multiply.
- The attention mask is analyzed on the host into a per-128x128-tile
  schedule (skip / full / mixed); mixed tiles multiply 0/1 tiles on GpSimd.
"""
import sys
import types
import numpy as np

B, S, DIM = 2, 2048, 1024
H, KVH, HD = 16, 4, 64
EPS = 1e-6
SCALE = HD ** -0.5
PT_TILES = S // 128  # 16
N_CHUNK = 512
N_CHUNKS = S // N_CHUNK  # 4

_BUILD_CACHE = {}


def _analyze_mask(mask):
    """Classify 128x128 tiles: 0=skip, 1=full, 2=mixed. Returns status grid,
    mixed tile stack (transposed to (k,q) layout, 0/1 float32), and index map.
    Index 0 of the stack is always the all-zero tile."""
    T = PT_TILES
    status = np.zeros((T, T), np.int8)
    tiles = [np.zeros((128, 128), np.float32)]
    idx = {}
    m = np.asarray(mask)
    for i in range(T):
        for j in range(T):
            sub = m[i * 128:(i + 1) * 128, j * 128:(j + 1) * 128]
            if not sub.any():
                status[i, j] = 0
            elif sub.all():
                status[i, j] = 1
            else:
                status[i, j] = 2
                idx[(i, j)] = len(tiles)
                tiles.append(np.ascontiguousarray(sub.T).astype(np.float32))
    return status, np.stack(tiles), idx


def _make_schedule(status, idx):
    """Per chunk: list of (ktile j, s0, s1, [(subtile s, mask_tile_index)])
    where [s0*128, s1*128) is the contiguous span of alive q-subtiles and the
    list holds per-subtile multiplies (zero tile for dead-in-span, mixed id
    for partial)."""
    sched = []
    for ci in range(N_CHUNKS):
        qts = list(range(4 * ci, 4 * ci + 4))
        entries = []
        for j in range(PT_TILES):
            st = [status[i, j] for i in qts]
            if not any(st):
                continue
            alive = [s for s in range(4) if st[s] != 0]
            s0, s1 = alive[0], alive[-1] + 1
            mults = []
            for s in range(s0, s1):
                if st[s] == 1:
                    continue
                mults.append((s, 0 if st[s] == 0 else idx[(qts[s], j)]))
            entries.append((j, s0, s1, mults))
        sched.append(entries)
    return sched


def _build(sched_key, sched, n_masks, neg_c):
    import concourse.bacc as bacc
    import concourse.mybir as mybir
    import concourse.tile as tile
    from concourse.masks import make_identity

    F32 = mybir.dt.float32
    F32R = mybir.dt.float32r

    nc = bacc.Bacc("TRN2", target_bir_lowering=False, debug=False)
    xT = nc.dram_tensor("xT", (DIM, S), F32R, kind="ExternalInput").ap()
    wq = nc.dram_tensor("wq", (DIM, 256), F32R, kind="ExternalInput").ap()
    wkv = nc.dram_tensor("wkv", (DIM, 128), F32R, kind="ExternalInput").ap()
    wo = nc.dram_tensor("wo", (256, DIM), F32R, kind="ExternalInput").ap()
    cosq = nc.dram_tensor("cosq", (128, S), F32, kind="ExternalInput").ap()
    sinq = nc.dram_tensor("sinq", (128, S), F32, kind="ExternalInput").ap()
    cosk = nc.dram_tensor("cosk", (64, S), F32, kind="ExternalInput").ap()
    sink = nc.dram_tensor("sink", (64, S), F32, kind="ExternalInput").ap()
    masks = nc.dram_tensor("masks", (n_masks, 128, 128), F32R,
                           kind="ExternalInput").ap()
    outT = nc.dram_tensor("outT", (DIM, S), F32, kind="ExternalOutput").ap()

    with tile.TileContext(nc) as tc:
        with tc.tile_pool(name="persist", bufs=1) as pp:
            # --- persistent tiles -------------------------------------
            wq_sb = pp.tile([128, 8, 256], F32R)
            nc.sync.dma_start(out=wq_sb, in_=wq.rearrange("(k p) m -> p k m", p=128))
            wkv_sb = pp.tile([128, 8, 128], F32R)
            nc.sync.dma_start(out=wkv_sb, in_=wkv.rearrange("(k p) m -> p k m", p=128))
            wo_sb = pp.tile([128, 2, DIM], F32R)
            nc.sync.dma_start(out=wo_sb, in_=wo.rearrange("(k p) m -> p k m", p=128))
            masks_sb = pp.tile([128, n_masks, 128], F32R)
            nc.sync.dma_start(out=masks_sb, in_=masks.rearrange("n k q -> k n q"))

            qtf = [pp.tile([128, S], F32R, tag=f"qtf{m}", name=f"qtf{m}") for m in range(2)]
            kt2 = pp.tile([128, S], F32R)
            v_aug = pp.tile([128, PT_TILES, 65], F32R)
            attn = [pp.tile([128, S], F32R, tag=f"attn{m}", name=f"attn{m}") for m in range(2)]

            ones1 = pp.tile([128, 1], F32)
            nc.vector.memset(ones1, 1.0)
            nc.vector.tensor_copy(v_aug[:, :, 64:65],
                                  ones1[:].broadcast_to([128, PT_TILES, 1]))
            # block-diagonal ones for the per-head sum-of-squares matmuls
            oq_f = pp.tile([128, 2], F32)
            nc.vector.memset(oq_f, 0.0)
            nc.vector.memset(oq_f[0:64, 0:1], 1.0)
            nc.vector.memset(oq_f[64:128, 1:2], 1.0)
            onesq = pp.tile([128, 2], F32R)
            nc.vector.tensor_copy(onesq[:], oq_f[:])
            ok_f = pp.tile([64, 1], F32)
            nc.vector.memset(ok_f, 1.0)
            onesk = pp.tile([64, 1], F32R)
            nc.vector.tensor_copy(onesk[:], ok_f[:])
            ident = pp.tile([64, 64], F32)
            make_identity(nc, ident[:])
            eps2 = pp.tile([2, 1], F32)
            nc.vector.memset(eps2, EPS)
            bias_c = pp.tile([128, 1], F32)
            nc.vector.memset(bias_c, neg_c)

            # --- phase 1: projections + rmsnorm + rope ----------------
            with tc.tile_pool(name="p1", bufs=2) as p1, \
                 tc.tile_pool(name="ps1", bufs=2, space="PSUM") as ps1:
                for ci in range(N_CHUNKS):
                    off = ci * N_CHUNK
                    xt = p1.tile([128, 8, N_CHUNK], F32R, tag="xt")
                    nc.sync.dma_start(
                        out=xt,
                        in_=xT[:, off:off + N_CHUNK].rearrange("(k p) n -> p k n", p=128))
                    cq = p1.tile([128, N_CHUNK], F32, tag="cq")
                    nc.sync.dma_start(out=cq, in_=cosq[:, off:off + N_CHUNK])
                    sq = p1.tile([128, N_CHUNK], F32, tag="sq")
                    nc.sync.dma_start(out=sq, in_=sinq[:, off:off + N_CHUNK])
                    ck = p1.tile([64, N_CHUNK], F32, tag="ck")
                    nc.sync.dma_start(out=ck, in_=cosk[:, off:off + N_CHUNK])
                    sk = p1.tile([64, N_CHUNK], F32, tag="sk")
                    nc.sync.dma_start(out=sk, in_=sink[:, off:off + N_CHUNK])

                    for m in range(2):
                        q_ps = ps1.tile([128, N_CHUNK], F32, tag="prj")
                        for k in range(8):
                            nc.tensor.matmul(q_ps[:],
                                             wq_sb[:, k, m * 128:(m + 1) * 128],
                                             xt[:, k, :],
                                             start=(k == 0), stop=(k == 7))
                        qtr = p1.tile([128, N_CHUNK], F32, tag="qtr")
                        nc.vector.tensor_copy(qtr[:], q_ps[:])
                        sqq = p1.tile([128, N_CHUNK], F32R, tag="sqq")
                        nc.vector.tensor_mul(sqq[:], qtr[:], qtr[:])
                        nrm_ps = ps1.tile([2, N_CHUNK], F32, tag="nrm")
                        nc.tensor.matmul(nrm_ps[:], onesq[:], sqq[:],
                                         start=True, stop=True)
                        nsb = p1.tile([2, N_CHUNK], F32, tag="nsb")
                        nc.scalar.activation(out=nsb[:], in_=nrm_ps[:],
                                             func=mybir.ActivationFunctionType.Sqrt,
                                             bias=eps2[:], scale=1.0 / HD)
                        nc.vector.reciprocal(nsb[:], nsb[:])
                        rep = p1.tile([128, N_CHUNK], F32, tag="rep")
                        nc.gpsimd.partition_broadcast(rep[0:64, :], nsb[0:1, :],
                                                      channels=64)
                        rb0 = p1.tile([1, N_CHUNK], F32, tag="rb0")
                        nc.sync.dma_start(out=rb0[:], in_=nsb[1:2, :])
                        bcb = p1.tile([64, N_CHUNK], F32, tag="bcb")
                        nc.gpsimd.partition_broadcast(bcb[:], rb0[:], channels=64)
                        nc.sync.dma_start(out=rep[64:128, :], in_=bcb[:])
                        qrot = p1.tile([128, N_CHUNK], F32, tag="qrot")
                        for blk, src in enumerate((32, 0, 96, 64)):
                            nc.sync.dma_start(out=qrot[blk * 32:(blk + 1) * 32, :],
                                              in_=qtr[src:src + 32, :])
                        t1 = p1.tile([128, N_CHUNK], F32, tag="t1")
                        nc.vector.tensor_mul(t1[:], qtr[:], cq[:])
                        t2 = p1.tile([128, N_CHUNK], F32, tag="t2")
                        nc.vector.tensor_mul(t2[:], qrot[:], sq[:])
                        nc.vector.tensor_add(t1[:], t1[:], t2[:])
                        nc.vector.tensor_mul(qtf[m][:, off:off + N_CHUNK],
                                             t1[:], rep[:])

                    kv_ps = ps1.tile([128, N_CHUNK], F32, tag="prj")
                    for k in range(8):
                        nc.tensor.matmul(kv_ps[:], wkv_sb[:, k, :], xt[:, k, :],
                                         start=(k == 0), stop=(k == 7))
                    ktr = p1.tile([64, N_CHUNK], F32, tag="ktr")
                    nc.vector.tensor_copy(ktr[:], kv_ps[0:64, :])
                    vtr = p1.tile([64, N_CHUNK], F32, tag="vtr")
                    nc.vector.tensor_copy(vtr[:], kv_ps[64:128, :])
                    sqk = p1.tile([64, N_CHUNK], F32R, tag="sqk")
                    nc.vector.tensor_mul(sqk[:], ktr[:], ktr[:])
                    nk_ps = ps1.tile([1, N_CHUNK], F32, tag="nrmk")
                    nc.tensor.matmul(nk_ps[:], onesk[:], sqk[:], start=True, stop=True)
                    nkb = p1.tile([1, N_CHUNK], F32, tag="nkb")
                    nc.scalar.activation(out=nkb[:], in_=nk_ps[:],
                                         func=mybir.ActivationFunctionType.Sqrt,
                                         bias=eps2[0:1, :], scale=1.0 / HD)
                    nc.vector.reciprocal(nkb[:], nkb[:])
                    krep = p1.tile([64, N_CHUNK], F32, tag="krep")
                    nc.gpsimd.partition_broadcast(krep[:], nkb[:], channels=64)
                    krot = p1.tile([64, N_CHUNK], F32, tag="krot")
                    nc.sync.dma_start(out=krot[0:32, :], in_=ktr[32:64, :])
                    nc.sync.dma_start(out=krot[32:64, :], in_=ktr[0:32, :])
                    k1 = p1.tile([64, N_CHUNK], F32, tag="k1")
                    nc.vector.tensor_mul(k1[:], ktr[:], ck[:])
                    k2 = p1.tile([64, N_CHUNK], F32, tag="k2")
                    nc.vector.tensor_mul(k2[:], krot[:], sk[:])
                    nc.vector.tensor_add(k1[:], k1[:], k2[:])
                    nc.vector.tensor_mul(kt2[0:64, off:off + N_CHUNK], k1[:], krep[:])
                    nc.sync.dma_start(out=kt2[64:128, off:off + N_CHUNK],
                                      in_=kt2[0:64, off:off + N_CHUNK])
                    for t in range(4):
                        j = 4 * ci + t
                        tr_ps = ps1.tile([128, 64], F32, tag="tr")
                        nc.tensor.transpose(tr_ps[:], vtr[:, t * 128:(t + 1) * 128],
                                            ident[:])
                        nc.vector.tensor_copy(v_aug[:, j, 0:64], tr_ps[:])

            # --- phase 2: attention -----------------------------------
            with tc.tile_pool(name="p2", bufs=6) as p2, \
                 tc.tile_pool(name="p2s", bufs=2) as p2s, \
                 tc.tile_pool(name="ps2", bufs=2, space="PSUM") as ps2, \
                 tc.tile_pool(name="psv", bufs=4, space="PSUM") as psv:
                for m in range(2):
                    for ci in range(N_CHUNKS):
                        off = ci * N_CHUNK
                        entries = sched[ci]
                        pv = [psv.tile([65, N_CHUNK], F32, tag="pv", name=f"pv{m}_{ci}_{_hh}") for _hh in range(2)]
                        for idx_e, (j, s0, s1, mults) in enumerate(entries):
                            koff = j * 128
                            a, b_ = s0 * 128, s1 * 128
                            st = ps2.tile([128, 2, N_CHUNK], F32, tag="st")
                            nc.tensor.matmul(
                                st[:, 0, a:b_],
                                kt2[0:64, koff:koff + 128],
                                qtf[m][0:64, off + a:off + b_],
                                start=True, stop=True)
                            nc.tensor.matmul(
                                st[:, 1, a:b_],
                                kt2[64:128, koff:koff + 128],
                                qtf[m][64:128, off + a:off + b_],
                                start=True, stop=True, tile_position=(64, 0))
                            pt = p2.tile([128, 2, N_CHUNK], F32R, tag="pt")
                            nc.scalar.activation(
                                out=pt[:, :, a:b_], in_=st[:, :, a:b_],
                                func=mybir.ActivationFunctionType.Exp,
                                bias=bias_c[:], scale=1.0)
                            for s, mt in mults:
                                for hh in range(2):
                                    nc.gpsimd.tensor_mul(
                                        pt[:, hh, s * 128:(s + 1) * 128],
                                        pt[:, hh, s * 128:(s + 1) * 128],
                                        masks_sb[:, mt, :])
                            first = (idx_e == 0)
                            last = (idx_e == len(entries) - 1)
                            for hh in range(2):
                                nc.tensor.matmul(pv[hh][:, a:b_],
                                                 v_aug[:, j, :],
                                                 pt[:, hh, a:b_],
                                                 start=first, stop=last)
                        for hh in range(2):
                            rd = p2s.tile([1, N_CHUNK], F32, tag="rd")
                            nc.vector.reciprocal(rd[:], pv[hh][64:65, :])
                            bcd = p2s.tile([64, N_CHUNK], F32, tag="bcd")
                            nc.gpsimd.partition_broadcast(bcd[:], rd[:], channels=64)
                            nc.vector.tensor_mul(
                                attn[m][hh * 64:(hh + 1) * 64, off:off + N_CHUNK],
                                pv[hh][0:64, :], bcd[:])

            # --- phase 3: out-projection ------------------------------
            with tc.tile_pool(name="p3", bufs=4) as p3, \
                 tc.tile_pool(name="ps3", bufs=2, space="PSUM") as ps3:
                for mo in range(8):
                    for ci in range(N_CHUNKS):
                        off = ci * N_CHUNK
                        o_ps = ps3.tile([128, N_CHUNK], F32, tag="ops")
                        for k2 in range(2):
                            nc.tensor.matmul(o_ps[:],
                                             wo_sb[:, k2, mo * 128:(mo + 1) * 128],
                                             attn[k2][:, off:off + N_CHUNK],
                                             start=(k2 == 0), stop=(k2 == 1))
                        o_sb = p3.tile([128, N_CHUNK], F32, tag="osb")
                        if (mo + ci) % 2 == 0:
                            nc.vector.tensor_copy(o_sb[:], o_ps[:])
                        else:
                            nc.scalar.copy(o_sb[:], o_ps[:])
                        nc.sync.dma_start(
                            out=outT[mo * 128:(mo + 1) * 128, off:off + N_CHUNK],
                            in_=o_sb[:])

    nc.compile()
    return nc


def _get_nc(sched_key, sched, n_masks, neg_c):
    key = (sched_key, n_masks, float(neg_c))
    if key not in _BUILD_CACHE:
        _BUILD_CACHE[key] = _build(sched_key, sched, n_masks, neg_c)
    return _BUILD_CACHE[key]


def kernel(x, Wq, Wkv, Wo, q_norm_w, k_norm_w, rope_cos, rope_sin,
           attention_mask):
    x = np.asarray(x, dtype=np.float32)
    Wq = np.asarray(Wq, dtype=np.float32)
    Wkv = np.asarray(Wkv, dtype=np.float32)
    Wo = np.asarray(Wo, dtype=np.float32)
    qw = np.asarray(q_norm_w, dtype=np.float32)
    kw = np.asarray(k_norm_w, dtype=np.float32)
    cos = np.asarray(rope_cos, dtype=np.float32)
    sin = np.asarray(rope_sin, dtype=np.float32)

    status, mask_tiles, idx = _analyze_mask(attention_mask)
    sched = _make_schedule(status, idx)
    sched_key = status.tobytes()

    # numerically safe exp shift (0 in the normal regime)
    mct_q = max(np.abs(cos).max(), np.abs(sin).max(), 1e-9)
    bound = SCALE * 2.0 * HD * mct_q * mct_q \
        * max(np.abs(qw).max(), 1e-9) * max(np.abs(kw).max(), 1e-9)
    neg_c = -max(0.0, float(bound) - 60.0)

    nc = _get_nc(sched_key, sched, mask_tiles.shape[0], neg_c)

    # host-folded rope tables (transposed layout, head-dim on partitions)
    half = HD // 2
    swap = np.concatenate([np.arange(half, HD), np.arange(0, half)])
    sgn = np.concatenate([-np.ones(half, np.float32), np.ones(half, np.float32)])
    cosq_h = (cos.T * qw[:, None] * SCALE).astype(np.float32)          # (64, S)
    sinq_h = (sin.T * (sgn * qw[swap])[:, None] * SCALE).astype(np.float32)
    cosk_h = (cos.T * kw[:, None]).astype(np.float32)
    sink_h = (sin.T * (sgn * kw[swap])[:, None]).astype(np.float32)
    cosq2 = np.ascontiguousarray(np.concatenate([cosq_h, cosq_h], axis=0))
    sinq2 = np.ascontiguousarray(np.concatenate([sinq_h, sinq_h], axis=0))

    in_maps = []
    for c in range(8):
        b, g = c // 4, c % 4
        im = {
            "xT": np.ascontiguousarray(x[b].T),
            "wq": np.ascontiguousarray(Wq[:, g * 256:(g + 1) * 256]),
            "wkv": np.ascontiguousarray(
                np.concatenate([Wkv[:, g * HD:(g + 1) * HD],
                                Wkv[:, KVH * HD + g * HD: KVH * HD + (g + 1) * HD]],
                               axis=1)),
            "wo": np.ascontiguousarray(Wo[g * 256:(g + 1) * 256, :]),
            "cosq": cosq2, "sinq": sinq2,
            "cosk": np.ascontiguousarray(cosk_h),
            "sink": np.ascontiguousarray(sink_h),
            "masks": mask_tiles,
        }
        in_maps.append(im)

    from concourse.bass_utils import run_bass_kernel_spmd
    res = run_bass_kernel_spmd(nc, in_maps, core_ids=list(range(8)), trace=False)

    out = np.zeros((B, S, DIM), dtype=np.float32)
    for c in range(8):
        out[c // 4] += res.results[c]["outT"].T
    return out
